# revision 2
# baseline (speedup 1.0000x reference)
"""Trainium2 Bass kernel v2 for nn_CNNEmbedding.

Structure (per core, data-parallel over tokens; 1024 tokens/core):
  - Gather split into channel-groups x 2 token-halves; each dma_gather moves
    512 tokens x group-channels into [128, gc, 512] bf16 channel-major tiles.
  - BN1 stats per (chunk, half) split across ACT (identity/square accum) and
    DVE (bn_stats), pipelined behind the gathers.
  - AllReduce of [2, 4352] f32 stats; no warmup AR (knob).
  - scale/shift; GELU1 in place per (chunk, half); matmul c-outer accumulating
    4 PSUM banks; tail chunks e-outer so BN2 stats overlap the last matmuls.
  - AR2 on [2, 512] stats; GELU2 from PSUM -> bf16 out tiles -> DMA out.
"""

import numpy as np
import ml_dtypes

BF16 = ml_dtypes.bfloat16

VOCAB = 8192
INTER = 4352
EMB = 512
N = 8192
NCORES = 8
NT = N // NCORES          # 1024 tokens per core
PT = 512                  # tokens per gather piece (hard ucode limit)
TH = NT // PT             # 2 token halves
C = INTER // 128          # 34 channel chunks
CE = EMB // 128           # 4 emb chunks
EPS = 1e-5

# ---- tuning knobs ----
GROUPS = [(17, (0, 1)), (17, (0, 1))]  # (chunks, queue per token-half)
ACT_FRAC = 0.26          # fraction of each group's chunks on ACT stats
CSPLIT_TAIL = 4          # last chunks run e-outer for BN2 stats overlap
OUT_BF16 = True
WARM_AR = True
WARM_GATHER = False       # tiny dummy gather to preload the Q7 ucode early
DELAY_W2T = True         # start the W2 load only after gathers are issued
MM1024 = False           # [128, 2, 512] rhs AP fails ISA s3d3_mm_num_elements

assert sum(g for g, _ in GROUPS) == C

_CACHE = {}


def _build_program():
    if "nc" in _CACHE:
        return _CACHE["nc"]

    import concourse.bacc as bacc
    from concourse import mybir, tile

    f32 = mybir.dt.float32
    bf16 = mybir.dt.bfloat16
    i16 = mybir.dt.int16
    AF = mybir.ActivationFunctionType
    ALU = mybir.AluOpType
    AX = mybir.AxisListType

    NQ = max(max(qs) for _, qs in GROUPS) + 1
    nc = bacc.Bacc("TRN2", target_bir_lowering=False, debug=False,
                   num_devices=NCORES, num_swdge_queues=NQ,
                   dynamic_dma_scratch_size=32768)

    out_dt = bf16 if OUT_BF16 else f32
    table = nc.dram_tensor("table", [VOCAB, INTER], bf16, kind="ExternalInput")
    idx = nc.dram_tensor("idx", [128, NT // 16], i16, kind="ExternalInput")
    w2t = nc.dram_tensor("w2t", [128, C, EMB], bf16, kind="ExternalInput")
    gb1 = nc.dram_tensor("gb1", [128, 2, C], f32, kind="ExternalInput")
    gb2 = nc.dram_tensor("gb2", [128, 2, CE], f32, kind="ExternalInput")
    out = nc.dram_tensor("out", [128, CE, NT], out_dt, kind="ExternalOutput")

    RG = [list(range(NCORES))]

    # group -> global chunk offsets, ACT/DVE split (ACT chunks first)
    c0s = []
    c0 = 0
    for gc, _ in GROUPS:
        c0s.append(c0)
        c0 += gc
    nacts = [round(ACT_FRAC * gc) for gc, _ in GROUPS]
    nbn = sum(gc - na for (gc, _), na in zip(GROUPS, nacts))

    with tile.TileContext(nc) as tc:
        with (
            tc.tile_pool(name="sb", bufs=1) as sb,
            tc.tile_pool(name="ps", bufs=1, space="PSUM") as ps,
            tc.tile_pool(name="dram", bufs=1, space="DRAM") as dram,
        ):
            idx_sb = sb.tile([128, NT // 16], i16, tag="idx", name="idx")
            nc.sync.dma_start(idx_sb[:], idx[:])
            gb1_sb = sb.tile([128, 2, C], f32, tag="gb1", name="gb1")
            gb2_sb = sb.tile([128, 2, CE], f32, tag="gb2", name="gb2")
            nc.scalar.dma_start(gb1_sb[:], gb1[:])
            nc.scalar.dma_start(gb2_sb[:], gb2[:])
            w2t_sb = sb.tile([128, C, EMB], bf16, tag="w2t", name="w2t")

            if WARM_GATHER:
                # absorb the gather ucode LOAD_LIB + first-call setup (~20us)
                # before the real index data even arrives.
                idxz = sb.tile([128, 32], i16, tag="idxz", name="idxz")
                hz = sb.tile([128, 1, PT], bf16, tag="hz", name="hz")
                nc.gpsimd.memset(idxz[:], 0)
                nc.gpsimd.dma_gather(
                    out_ap=hz[:],
                    in_ap=table[:, 0:128],
                    idxs_ap=idxz[:],
                    num_idxs=PT,
                    num_idxs_reg=PT,
                    elem_size=128,
                    elem_step=INTER,
                    transpose=True,
                    queue_num=0,
                )

            if WARM_AR:
                warm_i = dram.tile([128, 1], f32, tag="warm_i", name="warm_i")
                warm_o = dram.tile([128, 1], f32, tag="warm_o", name="warm_o")
                warm_sb = sb.tile([128, 1], f32, tag="warm_sb", name="warm_sb")
                nc.gpsimd.memset(warm_sb[:], 0.0)
                nc.sync.dma_start(warm_i[:], warm_sb[:])
                nc.gpsimd.collective_compute(
                    "AllReduce", ALU.add, replica_groups=RG,
                    ins=[warm_i.opt()], outs=[warm_o.opt()],
                )

            # ---- phase 1: gathers (channel-group x token-half) ----
            # h[g] is [128, TH, gc, PT]: each gather piece writes the
            # contiguous [128, gc, PT] block of its token half.
            hg = {}
            htile = {}
            for g, (gc, qs) in enumerate(GROUPS):
                h = sb.tile([128, TH, gc, PT], bf16, tag=f"h{g}", name=f"h{g}")
                htile[g] = h
                for t in range(TH):
                    nc.gpsimd.dma_gather(
                        out_ap=h[:, t, :, :],
                        in_ap=table[:, c0s[g] * 128:(c0s[g] + gc) * 128],
                        idxs_ap=idx_sb[:, t * (PT // 16):(t + 1) * (PT // 16)],
                        num_idxs=PT,
                        num_idxs_reg=PT,
                        elem_size=gc * 128,
                        elem_step=INTER,
                        transpose=True,
                        queue_num=qs[t % len(qs)],
                    )
                    hg[(g, t)] = h[:, t, :, :]

            if DELAY_W2T:
                # tiny gpsimd write into w2t_sb AFTER the gathers are queued:
                # the real w2t DMA (WAW on the tile) then starts only once the
                # gather descriptor generation is done, keeping the HBM
                # bandwidth free for the gather itself.
                nc.gpsimd.memset(w2t_sb[:, 0, 0:1], 0)
            nc.sync.dma_start(w2t_sb[:], w2t[:])

            # ---- BN1 stats per (chunk, half), per-group tiles ----
            # per group: ACT chunks (first na) accumulate into asum[g][t];
            # DVE chunks bn_stats into st6[g]; converted+merged into
            # arin1[g]; each group gets its own AllReduce so group 0's
            # normalize/matmuls overlap group 1's collective.
            trash = sb.tile([128, PT], bf16, tag="trash", name="trash")
            asum = {}
            st6 = {}
            arin1 = {}
            for g, (gc, _) in enumerate(GROUPS):
                na = nacts[g]
                nb = gc - na
                arin1[g] = sb.tile([128, 2, gc], f32, tag=f"arin1_{g}",
                                   name=f"arin1_{g}")
                if na:
                    asum[g] = [
                        sb.tile([128, 2, na], f32, tag=f"asum{g}_{t}",
                                name=f"asum{g}_{t}") for t in range(TH)]
                if nb:
                    st6[g] = sb.tile([128, nb, TH, 6], f32, tag=f"st6_{g}",
                                     name=f"st6_{g}")
                for t in range(TH):
                    for j in range(gc):
                        if j < na:
                            nc.scalar.activation(
                                trash[:], htile[g][:, t, j, :], AF.Identity,
                                accum_out=asum[g][t][:, 0, j:j + 1],
                            )
                            nc.scalar.activation(
                                trash[:], htile[g][:, t, j, :], AF.Square,
                                accum_out=asum[g][t][:, 1, j:j + 1],
                            )
                        else:
                            nc.vector.bn_stats(
                                st6[g][:, j - na, t, :],
                                htile[g][:, t, j, :],
                            )

            # convert + merge + AllReduce per group
            arout1 = {}
            for g, (gc, _) in enumerate(GROUPS):
                na = nacts[g]
                nb = gc - na
                if na:
                    nc.vector.tensor_add(
                        arin1[g][:, :, 0:na], asum[g][0][:], asum[g][1][:])
                if nb:
                    cvt = sb.tile([128, 3, nb], f32, tag=f"cvt{g}",
                                  name=f"cvt{g}")
                    sacc = cvt[:, 0, :]; qacc = cvt[:, 1, :]; tmp = cvt[:, 2, :]
                    first = True
                    for t in range(TH):
                        for p in range(2):
                            tc_ = st6[g][:, :, t, 3 * p + 0]
                            tm = st6[g][:, :, t, 3 * p + 1]
                            tM = st6[g][:, :, t, 3 * p + 2]
                            if first:
                                nc.vector.tensor_mul(sacc, tc_, tm)
                                nc.vector.tensor_mul(qacc, sacc, tm)
                                nc.vector.tensor_add(qacc, qacc, tM)
                                first = False
                            else:
                                nc.vector.tensor_mul(tmp, tc_, tm)
                                nc.vector.tensor_add(sacc, sacc, tmp)
                                nc.vector.tensor_mul(tmp, tmp, tm)
                                nc.vector.tensor_add(qacc, qacc, tmp)
                                nc.vector.tensor_add(qacc, qacc, tM)
                    nc.vector.tensor_copy(arin1[g][:, 0, na:gc], sacc)
                    nc.vector.tensor_copy(arin1[g][:, 1, na:gc], qacc)

                ar_in = dram.tile([128, 2, gc], f32, tag=f"ar1i{g}",
                                  name=f"ar1i{g}")
                ar_out = dram.tile([128, 2, gc], f32, tag=f"ar1o{g}",
                                   name=f"ar1o{g}")
                nc.sync.dma_start(ar_in[:], arin1[g][:])
                nc.gpsimd.collective_compute(
                    "AllReduce", ALU.add, replica_groups=RG,
                    ins=[ar_in.opt()], outs=[ar_out.opt()],
                )
                ao = sb.tile([128, 2, gc], f32, tag=f"arout1_{g}",
                             name=f"arout1_{g}")
                nc.sync.dma_start(ao[:], ar_out[:])
                arout1[g] = ao

            # ---- scale/shift 1, per group ----
            scale1 = {}
            shift1 = {}
            for g, (gc, _) in enumerate(GROUPS):
                w = sb.tile([128, 5, gc], f32, tag=f"ssw{g}", name=f"ssw{g}")
                mean_ = w[:, 0, :]; var_ = w[:, 1, :]; mm_ = w[:, 2, :]
                rstd_ = w[:, 3, :]
                sc = sb.tile([128, gc], f32, tag=f"scale1_{g}",
                             name=f"scale1_{g}")
                sh = sb.tile([128, gc], f32, tag=f"shift1_{g}",
                             name=f"shift1_{g}")
                a = c0s[g]
                nc.vector.tensor_scalar_mul(mean_, arout1[g][:, 0, :], 1.0 / N)
                nc.vector.tensor_mul(mm_, mean_, mean_)
                nc.vector.scalar_tensor_tensor(
                    var_, arout1[g][:, 1, :], 1.0 / N, mm_,
                    op0=ALU.mult, op1=ALU.subtract,
                )
                nc.vector.tensor_scalar_add(var_, var_, EPS)
                nc.scalar.activation(rstd_, var_, AF.Sqrt)
                nc.vector.reciprocal(rstd_, rstd_)
                nc.vector.tensor_mul(mm_, rstd_, rstd_)
                nc.vector.tensor_mul(mm_, mm_, var_)
                nc.vector.tensor_scalar(
                    mm_, mm_, -0.5, 1.5, op0=ALU.mult, op1=ALU.add,
                )
                nc.vector.tensor_mul(rstd_, rstd_, mm_)
                nc.vector.tensor_mul(sc[:], gb1_sb[:, 0, a:a + gc], rstd_)
                nc.vector.tensor_mul(mm_, sc[:], mean_)
                nc.vector.tensor_sub(sh[:], gb1_sb[:, 1, a:a + gc], mm_)
                scale1[g] = sc
                shift1[g] = sh

            # ---- GELU1 (in place) + matmuls ----
            def piece(cg, t):
                for g, (gc, _) in enumerate(GROUPS):
                    if c0s[g] <= cg < c0s[g] + gc:
                        return htile[g][:, t, cg - c0s[g], :]
                raise AssertionError(cg)

            def piece2(cg):
                for g, (gc, _) in enumerate(GROUPS):
                    if c0s[g] <= cg < c0s[g] + gc:
                        return htile[g][:, :, cg - c0s[g], :]
                raise AssertionError(cg)

            ps_out = [ps.tile([128, NT], f32, tag=f"po{e}", name=f"po{e}")
                      for e in range(CE)]
            arin2 = sb.tile([128, 2, CE], f32, tag="arin2", name="arin2")
            trash2 = sb.tile([128, NT], bf16, tag="trash2", name="trash2")

            def gof(cg):
                for g, (gc, _) in enumerate(GROUPS):
                    if c0s[g] <= cg < c0s[g] + gc:
                        return g, cg - c0s[g]
                raise AssertionError(cg)

            for cg in range(C):
                g, j = gof(cg)
                for t in range(TH):
                    nc.scalar.activation(
                        piece(cg, t), piece(cg, t), AF.Gelu,
                        bias=shift1[g][:, j:j + 1], scale=scale1[g][:, j:j + 1],
                    )

            def do_mm(cg, e, start, stop):
                lhsT = w2t_sb[:, cg, e * 128:(e + 1) * 128]
                if MM1024:
                    nc.tensor.matmul(
                        ps_out[e][:], lhsT, piece2(cg), start=start, stop=stop)
                else:
                    for t in range(TH):
                        nc.tensor.matmul(
                            ps_out[e][:, t * PT:(t + 1) * PT],
                            lhsT, piece(cg, t), start=start, stop=stop)

            CSP = C - CSPLIT_TAIL
            for cg in range(CSP):
                for e in range(CE):
                    do_mm(cg, e, start=(cg == 0), stop=False)
            for e in range(CE):
                for cg in range(CSP, C):
                    do_mm(cg, e, start=False, stop=(cg == C - 1))
                nc.vector.reduce_sum(arin2[:, 0, e:e + 1], ps_out[e][:], axis=AX.X)
                nc.scalar.activation(
                    trash2[:], ps_out[e][:], AF.Square,
                    accum_out=arin2[:, 1, e:e + 1],
                )

            # ---- AR2 ----
            ar2_in = dram.tile([128, 2, CE], f32, tag="ar2i", name="ar2i")
            ar2_out = dram.tile([128, 2, CE], f32, tag="ar2o", name="ar2o")
            nc.sync.dma_start(ar2_in[:], arin2[:])
            nc.gpsimd.collective_compute(
                "AllReduce", ALU.add, replica_groups=RG,
                ins=[ar2_in.opt()], outs=[ar2_out.opt()],
            )
            arout2 = sb.tile([128, 2, CE], f32, tag="arout2", name="arout2")
            nc.sync.dma_start(arout2[:], ar2_out[:])

            mean2 = sb.tile([128, CE], f32, tag="mean2", name="mean2")
            var2 = sb.tile([128, CE], f32, tag="var2", name="var2")
            mm2 = sb.tile([128, CE], f32, tag="mm2", name="mm2")
            rstd2 = sb.tile([128, CE], f32, tag="rstd2", name="rstd2")
            scale2 = sb.tile([128, CE], f32, tag="scale2", name="scale2")
            shift2 = sb.tile([128, CE], f32, tag="shift2", name="shift2")
            nc.vector.tensor_scalar_mul(mean2[:], arout2[:, 0, :], 1.0 / N)
            nc.vector.tensor_mul(mm2[:], mean2[:], mean2[:])
            nc.vector.scalar_tensor_tensor(
                var2[:], arout2[:, 1, :], 1.0 / N, mm2[:],
                op0=ALU.mult, op1=ALU.subtract,
            )
            nc.vector.tensor_scalar_add(var2[:], var2[:], EPS)
            nc.scalar.activation(rstd2[:], var2[:], AF.Sqrt)
            nc.vector.reciprocal(rstd2[:], rstd2[:])
            nc.vector.tensor_mul(mm2[:], rstd2[:], rstd2[:])
            nc.vector.tensor_mul(mm2[:], mm2[:], var2[:])
            nc.vector.tensor_scalar(
                mm2[:], mm2[:], -0.5, 1.5, op0=ALU.mult, op1=ALU.add,
            )
            nc.vector.tensor_mul(rstd2[:], rstd2[:], mm2[:])
            nc.vector.tensor_mul(scale2[:], gb2_sb[:, 0, :], rstd2[:])
            nc.vector.tensor_mul(mm2[:], scale2[:], mean2[:])
            nc.vector.tensor_sub(shift2[:], gb2_sb[:, 1, :], mm2[:])

            # ---- final normalize + GELU, write emb-major output ----
            out_sb = sb.tile([128, CE, NT], out_dt, tag="out", name="out")
            for e in range(CE):
                nc.scalar.activation(
                    out_sb[:, e, :], ps_out[e][:], AF.Gelu,
                    bias=shift2[:, e:e + 1], scale=scale2[:, e:e + 1],
                )
                nc.sync.dma_start(out[:, e, :], out_sb[:, e, :])

    nc.compile()
    _CACHE["nc"] = nc
    return nc


def kernel(x_t, W1, b1, g1, beta1, W2, b2, g2, beta2):
    from concourse.bass_utils import run_bass_kernel_spmd

    nc = _build_program()

    # b1/b2 cancel inside the BatchNorms (mean subtraction), so they are unused.
    table = np.ascontiguousarray(np.asarray(W1, dtype=np.float32).T).astype(BF16)
    w2t = np.ascontiguousarray(
        np.asarray(W2, dtype=np.float32).reshape(EMB, C, 128).transpose(2, 1, 0)
    ).astype(BF16)
    g1r = np.asarray(g1, dtype=np.float32).reshape(C, 128).T
    b1r = np.asarray(beta1, dtype=np.float32).reshape(C, 128).T
    gb1 = np.ascontiguousarray(np.stack([g1r, b1r], axis=1))      # [128, 2, C]
    g2r = np.asarray(g2, dtype=np.float32).reshape(CE, 128).T
    b2r = np.asarray(beta2, dtype=np.float32).reshape(CE, 128).T
    gb2 = np.ascontiguousarray(np.stack([g2r, b2r], axis=1))      # [128, 2, CE]

    x = np.asarray(x_t).astype(np.int64)
    in_maps = []
    for i in range(NCORES):
        xl = x[i * NT:(i + 1) * NT].astype(np.int16)
        wrapped = xl.reshape(NT // 16, 16).T                      # [16, NT//16]
        idxm = np.ascontiguousarray(np.tile(wrapped, (8, 1)))     # [128, NT//16]
        in_maps.append(
            {"table": table, "idx": idxm, "w2t": w2t, "gb1": gb1, "gb2": gb2}
        )

    _CACHE["in_maps"] = in_maps
    res = run_bass_kernel_spmd(nc, in_maps, list(range(NCORES)))

    shards = []
    for i in range(NCORES):
        o = np.asarray(res.results[i]["out"], dtype=np.float32)   # [128, CE, NT]
        shards.append(o.transpose(2, 1, 0).reshape(NT, EMB))      # [NT, EMB]
    return np.ascontiguousarray(np.concatenate(shards, axis=0)).astype(np.float32)


# revision 5
# speedup vs baseline: 1.0073x; 1.0073x over previous
"""Trainium2 Bass kernel v2 for nn_CNNEmbedding.

Structure (per core, data-parallel over tokens; 1024 tokens/core):
  - Gather split into channel-groups x 2 token-halves; each dma_gather moves
    512 tokens x group-channels into [128, gc, 512] bf16 channel-major tiles.
  - BN1 stats per (chunk, half) split across ACT (identity/square accum) and
    DVE (bn_stats), pipelined behind the gathers.
  - AllReduce of [2, 4352] f32 stats; no warmup AR (knob).
  - scale/shift; GELU1 in place per (chunk, half); matmul c-outer accumulating
    4 PSUM banks; tail chunks e-outer so BN2 stats overlap the last matmuls.
  - AR2 on [2, 512] stats; GELU2 from PSUM -> bf16 out tiles -> DMA out.
"""

import numpy as np
import ml_dtypes

BF16 = ml_dtypes.bfloat16

VOCAB = 8192
INTER = 4352
EMB = 512
N = 8192
NCORES = 8
NT = N // NCORES          # 1024 tokens per core
PT = 512                  # tokens per gather piece (hard ucode limit)
TH = NT // PT             # 2 token halves
C = INTER // 128          # 34 channel chunks
CE = EMB // 128           # 4 emb chunks
EPS = 1e-5

# ---- tuning knobs ----
GROUPS = [(17, (0, 1)), (17, (0, 1))]  # (chunks, queue per token-half)
ACT_FRAC = 0.26          # fraction of each group's chunks on ACT stats
CSPLIT_TAIL = 4          # last chunks run e-outer for BN2 stats overlap
OUT_BF16 = True
WARM_AR = True
WARM_GATHER = False       # tiny dummy gather to preload the Q7 ucode early
DELAY_W2T = True         # start the W2 load only after gathers are issued
MM1024 = False           # [128, 2, 512] rhs AP fails ISA s3d3_mm_num_elements

assert sum(g for g, _ in GROUPS) == C

_CACHE = {}


def _build_program():
    if "nc" in _CACHE:
        return _CACHE["nc"]

    import concourse.bacc as bacc
    from concourse import mybir, tile

    f32 = mybir.dt.float32
    bf16 = mybir.dt.bfloat16
    i16 = mybir.dt.int16
    AF = mybir.ActivationFunctionType
    ALU = mybir.AluOpType
    AX = mybir.AxisListType

    NQ = max(max(qs) for _, qs in GROUPS) + 1
    nc = bacc.Bacc("TRN2", target_bir_lowering=False, debug=False,
                   num_devices=NCORES, num_swdge_queues=NQ,
                   dynamic_dma_scratch_size=32768)

    out_dt = bf16 if OUT_BF16 else f32
    table = nc.dram_tensor("table", [VOCAB, INTER], bf16, kind="ExternalInput")
    idx = nc.dram_tensor("idx", [128, NT // 16], i16, kind="ExternalInput")
    w2t = nc.dram_tensor("w2t", [128, C, EMB], bf16, kind="ExternalInput")
    gb1 = nc.dram_tensor("gb1", [128, 2, C], f32, kind="ExternalInput")
    gb2 = nc.dram_tensor("gb2", [128, 2, CE], f32, kind="ExternalInput")
    out = nc.dram_tensor("out", [128, CE, NT], out_dt, kind="ExternalOutput")

    RG = [list(range(NCORES))]

    # group -> global chunk offsets, ACT/DVE split (ACT chunks first)
    c0s = []
    c0 = 0
    for gc, _ in GROUPS:
        c0s.append(c0)
        c0 += gc
    nacts = [round(ACT_FRAC * gc) for gc, _ in GROUPS]
    nbn = sum(gc - na for (gc, _), na in zip(GROUPS, nacts))

    with tile.TileContext(nc) as tc:
        with (
            tc.tile_pool(name="sb", bufs=1) as sb,
            tc.tile_pool(name="ps", bufs=1, space="PSUM") as ps,
            tc.tile_pool(name="dram", bufs=1, space="DRAM") as dram,
        ):
            idx_sb = sb.tile([128, NT // 16], i16, tag="idx", name="idx")
            nc.sync.dma_start(idx_sb[:], idx[:])
            gb1_sb = sb.tile([128, 2, C], f32, tag="gb1", name="gb1")
            gb2_sb = sb.tile([128, 2, CE], f32, tag="gb2", name="gb2")
            nc.scalar.dma_start(gb1_sb[:], gb1[:])
            nc.scalar.dma_start(gb2_sb[:], gb2[:])
            w2t_sb = sb.tile([128, C, EMB], bf16, tag="w2t", name="w2t")

            if WARM_GATHER:
                # absorb the gather ucode LOAD_LIB + first-call setup (~20us)
                # before the real index data even arrives.
                idxz = sb.tile([128, 32], i16, tag="idxz", name="idxz")
                hz = sb.tile([128, 1, PT], bf16, tag="hz", name="hz")
                nc.gpsimd.memset(idxz[:], 0)
                nc.gpsimd.dma_gather(
                    out_ap=hz[:],
                    in_ap=table[:, 0:128],
                    idxs_ap=idxz[:],
                    num_idxs=PT,
                    num_idxs_reg=PT,
                    elem_size=128,
                    elem_step=INTER,
                    transpose=True,
                    queue_num=0,
                )

            if WARM_AR:
                warm_i = dram.tile([128, 1], f32, tag="warm_i", name="warm_i")
                warm_o = dram.tile([128, 1], f32, tag="warm_o", name="warm_o")
                warm_sb = sb.tile([128, 1], f32, tag="warm_sb", name="warm_sb")
                nc.gpsimd.memset(warm_sb[:], 0.0)
                nc.sync.dma_start(warm_i[:], warm_sb[:])
                nc.gpsimd.collective_compute(
                    "AllReduce", ALU.add, replica_groups=RG,
                    ins=[warm_i.opt()], outs=[warm_o.opt()],
                )

            # ---- phase 1: gathers (channel-group x token-half) ----
            # h[g] is [128, TH, gc, PT]: each gather piece writes the
            # contiguous [128, gc, PT] block of its token half.
            hg = {}
            htile = {}
            for g, (gc, qs) in enumerate(GROUPS):
                h = sb.tile([128, TH, gc, PT], bf16, tag=f"h{g}", name=f"h{g}")
                htile[g] = h
                for t in range(TH):
                    nc.gpsimd.dma_gather(
                        out_ap=h[:, t, :, :],
                        in_ap=table[:, c0s[g] * 128:(c0s[g] + gc) * 128],
                        idxs_ap=idx_sb[:, t * (PT // 16):(t + 1) * (PT // 16)],
                        num_idxs=PT,
                        num_idxs_reg=PT,
                        elem_size=gc * 128,
                        elem_step=INTER,
                        transpose=True,
                        queue_num=qs[t % len(qs)],
                    )
                    hg[(g, t)] = h[:, t, :, :]

            if DELAY_W2T:
                # tiny gpsimd write into w2t_sb AFTER the gathers are queued:
                # the real w2t DMA (WAW on the tile) then starts only once the
                # gather descriptor generation is done, keeping the HBM
                # bandwidth free for the gather itself.
                nc.gpsimd.memset(w2t_sb[:, 0, 0:1], 0)
            nc.sync.dma_start(w2t_sb[:], w2t[:])

            # ---- BN1 stats per (chunk, half), per-group tiles ----
            # per group: ACT chunks (first na) accumulate into asum[g][t];
            # DVE chunks bn_stats into st6[g]; converted+merged into
            # arin1[g]; each group gets its own AllReduce so group 0's
            # normalize/matmuls overlap group 1's collective.
            trash = sb.tile([128, PT], bf16, tag="trash", name="trash")
            asum = {}
            st6 = {}
            arin1 = {}
            for g, (gc, _) in enumerate(GROUPS):
                na = nacts[g]
                nb = gc - na
                arin1[g] = sb.tile([128, 2, gc], f32, tag=f"arin1_{g}",
                                   name=f"arin1_{g}")
                if na:
                    asum[g] = [
                        sb.tile([128, 2, na], f32, tag=f"asum{g}_{t}",
                                name=f"asum{g}_{t}") for t in range(TH)]
                if nb:
                    st6[g] = sb.tile([128, nb, TH, 6], f32, tag=f"st6_{g}",
                                     name=f"st6_{g}")
                for t in range(TH):
                    for j in range(gc):
                        if j < na:
                            nc.scalar.activation(
                                trash[:], htile[g][:, t, j, :], AF.Identity,
                                accum_out=asum[g][t][:, 0, j:j + 1],
                            )
                            nc.scalar.activation(
                                trash[:], htile[g][:, t, j, :], AF.Square,
                                accum_out=asum[g][t][:, 1, j:j + 1],
                            )
                        else:
                            nc.vector.bn_stats(
                                st6[g][:, j - na, t, :],
                                htile[g][:, t, j, :],
                            )

            # convert + merge + AllReduce per group
            arout1 = {}
            for g, (gc, _) in enumerate(GROUPS):
                na = nacts[g]
                nb = gc - na
                if na:
                    nc.vector.tensor_add(
                        arin1[g][:, :, 0:na], asum[g][0][:], asum[g][1][:])
                if nb:
                    cvt = sb.tile([128, 3, nb], f32, tag=f"cvt{g}",
                                  name=f"cvt{g}")
                    sacc = cvt[:, 0, :]; qacc = cvt[:, 1, :]; tmp = cvt[:, 2, :]
                    first = True
                    for t in range(TH):
                        for p in range(2):
                            tc_ = st6[g][:, :, t, 3 * p + 0]
                            tm = st6[g][:, :, t, 3 * p + 1]
                            tM = st6[g][:, :, t, 3 * p + 2]
                            if first:
                                nc.vector.tensor_mul(sacc, tc_, tm)
                                nc.vector.tensor_mul(qacc, sacc, tm)
                                nc.vector.tensor_add(qacc, qacc, tM)
                                first = False
                            else:
                                nc.vector.tensor_mul(tmp, tc_, tm)
                                nc.vector.tensor_add(sacc, sacc, tmp)
                                nc.vector.tensor_mul(tmp, tmp, tm)
                                nc.vector.tensor_add(qacc, qacc, tmp)
                                nc.vector.tensor_add(qacc, qacc, tM)
                    nc.vector.tensor_copy(arin1[g][:, 0, na:gc], sacc)
                    nc.vector.tensor_copy(arin1[g][:, 1, na:gc], qacc)

                ar_in = dram.tile([128, 2, gc], f32, tag=f"ar1i{g}",
                                  name=f"ar1i{g}")
                ar_out = dram.tile([128, 2, gc], f32, tag=f"ar1o{g}",
                                   name=f"ar1o{g}")
                nc.sync.dma_start(ar_in[:], arin1[g][:])
                nc.gpsimd.collective_compute(
                    "AllReduce", ALU.add, replica_groups=RG,
                    ins=[ar_in.opt()], outs=[ar_out.opt()],
                )
                ao = sb.tile([128, 2, gc], f32, tag=f"arout1_{g}",
                             name=f"arout1_{g}")
                nc.sync.dma_start(ao[:], ar_out[:])
                arout1[g] = ao

            # ---- scale/shift 1, per group ----
            scale1 = {}
            shift1 = {}
            for g, (gc, _) in enumerate(GROUPS):
                w = sb.tile([128, 5, gc], f32, tag=f"ssw{g}", name=f"ssw{g}")
                mean_ = w[:, 0, :]; var_ = w[:, 1, :]; mm_ = w[:, 2, :]
                rstd_ = w[:, 3, :]
                sc = sb.tile([128, gc], f32, tag=f"scale1_{g}",
                             name=f"scale1_{g}")
                sh = sb.tile([128, gc], f32, tag=f"shift1_{g}",
                             name=f"shift1_{g}")
                a = c0s[g]
                nc.vector.tensor_scalar_mul(mean_, arout1[g][:, 0, :], 1.0 / N)
                nc.vector.tensor_mul(mm_, mean_, mean_)
                nc.vector.scalar_tensor_tensor(
                    var_, arout1[g][:, 1, :], 1.0 / N, mm_,
                    op0=ALU.mult, op1=ALU.subtract,
                )
                nc.vector.tensor_scalar_add(var_, var_, EPS)
                nc.scalar.activation(rstd_, var_, AF.Sqrt)
                nc.vector.reciprocal(rstd_, rstd_)
                nc.vector.tensor_mul(mm_, rstd_, rstd_)
                nc.vector.tensor_mul(mm_, mm_, var_)
                nc.vector.tensor_scalar(
                    mm_, mm_, -0.5, 1.5, op0=ALU.mult, op1=ALU.add,
                )
                nc.vector.tensor_mul(rstd_, rstd_, mm_)
                nc.vector.tensor_mul(sc[:], gb1_sb[:, 0, a:a + gc], rstd_)
                nc.vector.tensor_mul(mm_, sc[:], mean_)
                nc.vector.tensor_sub(sh[:], gb1_sb[:, 1, a:a + gc], mm_)
                scale1[g] = sc
                shift1[g] = sh

            # ---- GELU1 (in place) + matmuls ----
            def piece(cg, t):
                for g, (gc, _) in enumerate(GROUPS):
                    if c0s[g] <= cg < c0s[g] + gc:
                        return htile[g][:, t, cg - c0s[g], :]
                raise AssertionError(cg)

            def piece2(cg):
                for g, (gc, _) in enumerate(GROUPS):
                    if c0s[g] <= cg < c0s[g] + gc:
                        return htile[g][:, :, cg - c0s[g], :]
                raise AssertionError(cg)

            ps_out = [ps.tile([128, NT], f32, tag=f"po{e}", name=f"po{e}")
                      for e in range(CE)]
            arin2 = sb.tile([128, 2, CE], f32, tag="arin2", name="arin2")
            trash2 = sb.tile([128, NT], bf16, tag="trash2", name="trash2")

            def gof(cg):
                for g, (gc, _) in enumerate(GROUPS):
                    if c0s[g] <= cg < c0s[g] + gc:
                        return g, cg - c0s[g]
                raise AssertionError(cg)

            for cg in range(C):
                g, j = gof(cg)
                for t in range(TH):
                    nc.scalar.activation(
                        piece(cg, t), piece(cg, t), AF.Gelu,
                        bias=shift1[g][:, j:j + 1], scale=scale1[g][:, j:j + 1],
                    )

            def do_mm(cg, e, start, stop):
                lhsT = w2t_sb[:, cg, e * 128:(e + 1) * 128]
                if MM1024:
                    nc.tensor.matmul(
                        ps_out[e][:], lhsT, piece2(cg), start=start, stop=stop)
                else:
                    for t in range(TH):
                        nc.tensor.matmul(
                            ps_out[e][:, t * PT:(t + 1) * PT],
                            lhsT, piece(cg, t), start=start, stop=stop)

            CSP = C - CSPLIT_TAIL
            for cg in range(CSP):
                for e in range(CE):
                    do_mm(cg, e, start=(cg == 0), stop=False)
            for e in range(CE):
                for cg in range(CSP, C):
                    do_mm(cg, e, start=False, stop=(cg == C - 1))
                nc.vector.reduce_sum(arin2[:, 0, e:e + 1], ps_out[e][:], axis=AX.X)
                nc.scalar.activation(
                    trash2[:], ps_out[e][:], AF.Square,
                    accum_out=arin2[:, 1, e:e + 1],
                )

            # ---- AR2 ----
            ar2_in = dram.tile([128, 2, CE], f32, tag="ar2i", name="ar2i")
            ar2_out = dram.tile([128, 2, CE], f32, tag="ar2o", name="ar2o")
            nc.sync.dma_start(ar2_in[:], arin2[:])
            nc.gpsimd.collective_compute(
                "AllReduce", ALU.add, replica_groups=RG,
                ins=[ar2_in.opt()], outs=[ar2_out.opt()],
            )
            arout2 = sb.tile([128, 2, CE], f32, tag="arout2", name="arout2")
            nc.sync.dma_start(arout2[:], ar2_out[:])

            mean2 = sb.tile([128, CE], f32, tag="mean2", name="mean2")
            var2 = sb.tile([128, CE], f32, tag="var2", name="var2")
            mm2 = sb.tile([128, CE], f32, tag="mm2", name="mm2")
            rstd2 = sb.tile([128, CE], f32, tag="rstd2", name="rstd2")
            scale2 = sb.tile([128, CE], f32, tag="scale2", name="scale2")
            shift2 = sb.tile([128, CE], f32, tag="shift2", name="shift2")
            nc.vector.tensor_scalar_mul(mean2[:], arout2[:, 0, :], 1.0 / N)
            nc.vector.tensor_mul(mm2[:], mean2[:], mean2[:])
            nc.vector.scalar_tensor_tensor(
                var2[:], arout2[:, 1, :], 1.0 / N, mm2[:],
                op0=ALU.mult, op1=ALU.subtract,
            )
            nc.vector.tensor_scalar_add(var2[:], var2[:], EPS)
            nc.scalar.activation(rstd2[:], var2[:], AF.Sqrt)
            nc.vector.reciprocal(rstd2[:], rstd2[:])
            nc.vector.tensor_mul(mm2[:], rstd2[:], rstd2[:])
            nc.vector.tensor_mul(mm2[:], mm2[:], var2[:])
            nc.vector.tensor_scalar(
                mm2[:], mm2[:], -0.5, 1.5, op0=ALU.mult, op1=ALU.add,
            )
            nc.vector.tensor_mul(rstd2[:], rstd2[:], mm2[:])
            nc.vector.tensor_mul(scale2[:], gb2_sb[:, 0, :], rstd2[:])
            nc.vector.tensor_mul(mm2[:], scale2[:], mean2[:])
            nc.vector.tensor_sub(shift2[:], gb2_sb[:, 1, :], mm2[:])

            # ---- final normalize + GELU, write emb-major output ----
            out_sb = sb.tile([128, CE, NT], out_dt, tag="out", name="out")
            for e in range(CE):
                nc.scalar.activation(
                    out_sb[:, e, :], ps_out[e][:], AF.Gelu,
                    bias=shift2[:, e:e + 1], scale=scale2[:, e:e + 1],
                )
                nc.sync.dma_start(out[:, e, :], out_sb[:, e, :])

    nc.compile()
    _CACHE["nc"] = nc
    return nc


def kernel(x_t, W1, b1, g1, beta1, W2, b2, g2, beta2):
    from concourse.bass_utils import run_bass_kernel_spmd

    nc = _build_program()

    # b1/b2 cancel inside the BatchNorms (mean subtraction), so they are unused.
    table = np.ascontiguousarray(np.asarray(W1, dtype=np.float32).T).astype(BF16)
    w2t = np.ascontiguousarray(
        np.asarray(W2, dtype=np.float32).reshape(EMB, C, 128).transpose(2, 1, 0)
    ).astype(BF16)
    g1r = np.asarray(g1, dtype=np.float32).reshape(C, 128).T
    b1r = np.asarray(beta1, dtype=np.float32).reshape(C, 128).T
    gb1 = np.ascontiguousarray(np.stack([g1r, b1r], axis=1))      # [128, 2, C]
    g2r = np.asarray(g2, dtype=np.float32).reshape(CE, 128).T
    b2r = np.asarray(beta2, dtype=np.float32).reshape(CE, 128).T
    gb2 = np.ascontiguousarray(np.stack([g2r, b2r], axis=1))      # [128, 2, CE]

    x = np.asarray(x_t).astype(np.int64)
    in_maps = []
    for i in range(NCORES):
        xl = x[i * NT:(i + 1) * NT].astype(np.int16)
        wrapped = xl.reshape(NT // 16, 16).T                      # [16, NT//16]
        idxm = np.ascontiguousarray(np.tile(wrapped, (8, 1)))     # [128, NT//16]
        in_maps.append(
            {"table": table, "idx": idxm, "w2t": w2t, "gb1": gb1, "gb2": gb2}
        )

    _CACHE["in_maps"] = in_maps
    res = run_bass_kernel_spmd(nc, in_maps, list(range(NCORES)))

    shards = []
    for i in range(NCORES):
        o = np.asarray(res.results[i]["out"], dtype=np.float32)   # [128, CE, NT]
        shards.append(o.transpose(2, 1, 0).reshape(NT, EMB))      # [NT, EMB]
    return np.ascontiguousarray(np.concatenate(shards, axis=0)).astype(np.float32)
